# revision 34
# baseline (speedup 1.0000x reference)
"""Causal self-attention (B=2, S=2048, D=1024, H=16) on 8 TRN2 NeuronCores.

Sharding: core c -> batch b = c//4, head group g = c%4 (4 heads each).
Each core computes the qkv projection for its heads, RoPE, causal flash
attention, and a partial out-projection (row-parallel); the host sums the
4 partials per batch.

Layout strategy (everything "transposed", seq on the free axis):
  xt  = x[b]^T                  [D, S]   bf16 (host-prepped)
  Qt/Kt[m, s] per head          computed as  W[:,m]^T @ xt  (lhsT = W slice)
  V natural [s, m]              computed as  xt_tile^T @ Wv
  St[k, q]  = Kt_tile^T @ Qt    -> exp -> causal mask -> Pt (bf16)
  Ot'[m+1, q] = [V|1]^T @ Pt    (row m==HD is the softmax denominator l[q])
  y^T[n, s] = Wo[:,n]^T @ (Ot/l)  accumulated over m tiles; host sums cores.

v4 structure (vs v3's 168us):
 - exp on the ACT engine is the largest serial resource (~84us); v3
   back-loaded it (chunk ci has ci+1 key tiles).  v4 emits attention
   pairs in order (2,0)(2,1)(3,0)(3,1)(1,*)(0,*) so the big chunks' exp
   starts ~12us in, and paces emission with a fine-grained scheduler:
   each QK+exp unit is followed by ~0.7us of PE filler (qkv projection
   mt-tiles, V sl-tiles, AV segments of the previous pair, out-proj
   halves) popped from a dependency-tagged queue.
 - causal mask: only the 128-col diagonal window of each tile needs
   masking; one [128,2,128] bf16 mul per diagonal tile (both heads at
   once) instead of the full-tail mul (saves ~25us of Vector).
 - epilogue: 1/l via reciprocal on the PSUM row directly, GpSimd
   broadcast, single fused (O * 1/l) mul from PSUM (saves the osb/l
   copies, ~22us of Vector).
 - inputs split across both HWDGE rings (sync + scalar), ordered so
   kt0/qt2/csn land first; 64 small warm matmuls un-throttle the PE
   HAM clock before the first real matmul.
"""

from contextlib import ExitStack

import numpy as np
import ml_dtypes

import concourse.bass as bass
import concourse.tile as tile
import concourse.mybir as mybir
from concourse import bacc
from concourse.bass_utils import run_bass_kernel_spmd

HD = 64          # head dim
CH = 512         # seq chunk (one PSUM bank of fp32)
_SHUF = [(i + 16) % 32 for i in range(32)]  # swap 16-halves in each quadrant


def rope_perm():
    """Within-head output-column permutation: local row r <- reference col."""
    perm = np.zeros(HD, dtype=np.int64)
    for r in range(HD):
        q, pos = divmod(r, 32)
        x2 = pos >= 16
        f = q * 16 + (pos % 16)
        perm[r] = 2 * f + (1 if x2 else 0)
    return perm


def rope_tables(rope_cos, rope_sin, S):
    """cos/sin tables [128, S] fp32 aligned with the permuted Qt/Kt rows."""
    cs = np.zeros((128, S), np.float32)
    sn = np.zeros((128, S), np.float32)
    for r in range(128):
        rr = r % HD
        q, pos = divmod(rr, 32)
        x2 = pos >= 16
        f = q * 16 + (pos % 16)
        cs[r] = rope_cos[:S, f]
        sn[r] = rope_sin[:S, f] * (-1.0 if x2 else 1.0)
    return cs, sn


def build_core(nc, S, D, HC):
    """Emit the per-core kernel IR. HC = heads on this core."""
    DT = D // 128           # contraction tiles over model dim
    M = HC * HD             # local qkv width
    MT = M // 128           # m tiles
    NCH = S // CH           # seq chunks
    KPC = CH // 128         # key tiles per chunk
    NT = D // 128           # out-proj n tiles
    NT2 = NT // 2
    HP = HC // 2            # head pairs
    fp32, bf16 = mybir.dt.float32, mybir.dt.bfloat16
    SCALE = float(HD) ** -0.5

    xt_d = nc.declare_dram_parameter("xt", [NCH, 128, DT * CH], bf16, isOutput=False)
    wq_d = nc.declare_dram_parameter("wq", [128, DT * M], bf16, isOutput=False)
    wk_d = nc.declare_dram_parameter("wk", [128, DT * M], bf16, isOutput=False)
    wv_d = nc.declare_dram_parameter("wv", [128, DT * M], bf16, isOutput=False)
    wo_d = nc.declare_dram_parameter("wo", [128, MT * D], bf16, isOutput=False)
    csn_d = nc.declare_dram_parameter("csn", [128, 2 * NCH * CH], bf16, isOutput=False)
    yt_d = nc.declare_dram_parameter("yt", [NCH, 2, 128, NT2 * CH], bf16, isOutput=True)

    with tile.TileContext(nc) as tc, ExitStack() as ctx:
        persist = ctx.enter_context(tc.tile_pool(name="persist", bufs=1))
        mm_ps = ctx.enter_context(tc.tile_pool(name="mm_ps", bufs=2, space="PSUM"))
        st_ps = ctx.enter_context(tc.tile_pool(name="st_ps", bufs=2, space="PSUM"))
        ot_ps = ctx.enter_context(tc.tile_pool(name="ot_ps", bufs=2, space="PSUM"))
        work = ctx.enter_context(tc.tile_pool(name="work", bufs=3))
        pt_pool = ctx.enter_context(tc.tile_pool(name="ptp", bufs=26))
        out_pool = ctx.enter_context(tc.tile_pool(name="outp", bufs=2))

        # ---- persistent tiles -------------------------------------------
        xt = [persist.tile([128, DT, CH], bf16, name=f"xt_{c}") for c in range(NCH)]
        wq = persist.tile([128, DT, M], bf16)
        wk = persist.tile([128, DT, M], bf16)
        wv = persist.tile([128, DT, M], bf16)
        wo = persist.tile([128, MT, D], bf16)
        # csn is chunk-major in DMA-arrival order [0, 2, 1, 3] (slot map
        # below); each slot holds [cos, sin] rows for one chunk.
        CSLOT = {0: 0, 2: 1, 1: 2, 3: 3}
        csn = persist.tile([128, NCH, 2, CH], bf16)
        qt = [persist.tile([128, MT, CH], bf16, name=f"qt_{c}") for c in range(NCH)]
        kt = [persist.tile([128, MT, CH], bf16, name=f"kt_{c}") for c in range(NCH)]
        vsb = [persist.tile([128, KPC, HC, HD + 1], bf16, name=f"vsb_{c}")
               for c in range(NCH)]
        otn = [persist.tile([128, MT, CH], bf16, name=f"otn_{c}") for c in range(NCH)]
        cmask = persist.tile([128, 2, 128], bf16)

        # ---- input DMAs split across the two HWDGE rings ----------------
        # ring1 (sync): wk, xt0, xt2, xt1, xt3, wo.  ring2 (scalar): csn in
        # two halves (host orders chunks [0,2,1,3] so the first half covers
        # the chunks attention touches first), wq, wv.  Attention starts on
        # chunk 0, which needs only wk+xt0+csn0+wq.
        nc.sync.dma_start(out=wk.rearrange("p t m -> p (t m)"), in_=wk_d[:, :])
        nc.scalar.dma_start(out=wq.rearrange("p t m -> p (t m)"), in_=wq_d[:, :])
        nc.sync.dma_start(
            out=xt[0].rearrange("p t s -> p (t s)"), in_=xt_d[0])
        nc.scalar.dma_start(
            out=csn[:, 0:2, :, :].rearrange("p c i s -> p (c i s)"),
            in_=csn_d[:, 0:NCH * CH])
        nc.sync.dma_start(
            out=xt[2].rearrange("p t s -> p (t s)"), in_=xt_d[2])
        nc.scalar.dma_start(
            out=csn[:, 2:4, :, :].rearrange("p c i s -> p (c i s)"),
            in_=csn_d[:, NCH * CH:])
        nc.scalar.dma_start(out=wv.rearrange("p t m -> p (t m)"), in_=wv_d[:, :])
        nc.sync.dma_start(
            out=xt[1].rearrange("p t s -> p (t s)"), in_=xt_d[1])
        nc.sync.dma_start(
            out=xt[3].rearrange("p t s -> p (t s)"), in_=xt_d[3])
        nc.sync.dma_start(out=wo.rearrange("p t n -> p (t n)"), in_=wo_d[:, :])

        # ---- PE warmup: ~5us of small matmuls un-throttle the HAM clock
        # while the DMAs stream (they depend only on the memsets).
        warm_w = persist.tile([128, 128], bf16)
        nc.vector.memset(warm_w[:], 0.0)
        # preload the exp table set (~2.7us) before the first real exp
        warm_act = work.tile([128, 16], fp32, tag="wact")
        nc.scalar.activation(out=warm_act[:], in_=warm_w[:, 0:16],
                             func=mybir.ActivationFunctionType.Exp)
        warm_ps = mm_ps.tile([128, CH], fp32, tag="mm", name="mmps")
        for _ in range(128):
            nc.tensor.matmul(warm_ps[:, 0:128], warm_w[:], warm_w[:],
                             start=True, stop=True)

        # causal mask for the 128-col diagonal window (both head slots):
        # keep j - p >= 0 (query-local j, key-local p).  Same for every
        # diagonal tile.  Built on GpSimd while it is otherwise idle.
        nc.gpsimd.memset(cmask[:], 1.0)
        for i in range(2):
            nc.gpsimd.affine_select(
                out=cmask[:, i, :], in_=cmask[:, i, :],
                compare_op=mybir.AluOpType.is_ge, fill=0.0,
                base=0, pattern=[[1, 128]], channel_multiplier=-1,
            )
        for c in range(NCH):
            nc.vector.memset(vsb[c][:, :, :, HD:HD + 1], 1.0)

        # ---- emission units ---------------------------------------------
        def qk_mt(ci, wt, dst, mt):
            """One mt-tile of a q/k projection + rope."""
            ps = mm_ps.tile([128, CH], fp32, tag="mm", name="mmps")
            for dt in range(DT):
                nc.tensor.matmul(
                    ps[:],
                    wt[:, dt, mt * 128:(mt + 1) * 128],
                    xt[ci][:, dt, :],
                    start=(dt == 0), stop=(dt == DT - 1),
                )
            # rope: 2 muls + partition shuffle + add.
            sl = CSLOT[ci]
            p1 = work.tile([128, CH], fp32, tag="p1")
            p2 = work.tile([128, CH], fp32, tag="p2")
            p2s = work.tile([128, CH], fp32, tag="p2s")
            nc.vector.tensor_mul(p1[:], ps[:], csn[:, sl, 0, :])
            nc.vector.tensor_mul(p2[:], ps[:], csn[:, sl, 1, :])
            nc.vector.stream_shuffle(p2s[:], p2[:], mask=_SHUF)
            nc.vector.tensor_add(dst[ci][:, mt, :], p1[:], p2s[:])

        def v_sl(ci, sl):
            """One 128-row slice of the V projection."""
            ps = mm_ps.tile([128, M], fp32, tag="mm", name="vps")
            for dt in range(DT):
                nc.tensor.matmul(
                    ps[:],
                    xt[ci][:, dt, sl * 128:(sl + 1) * 128],
                    wv[:, dt, :],
                    start=(dt == 0), stop=(dt == DT - 1),
                )
            nc.vector.tensor_copy(
                vsb[ci][:, sl, :, 0:HD],
                ps.rearrange("p (h d) -> p h d", h=HC),
            )

        pts_map = {}     # (ci, hp) -> list of (pt, trim)
        ots_map = {}     # (ci, hp) -> {h: psum tile}

        def qk_kj(ci, hp, kj):
            """QK scores for one key tile (both heads), exp, diag mask."""
            heads = (2 * hp, 2 * hp + 1)
            mt = hp
            tidx = kj - ci * KPC
            trim = max(0, tidx) * 128
            kc, kl = divmod(kj, KPC)
            stp = st_ps.tile([128, 2, CH], fp32, tag="st")
            for i, h in enumerate(heads):
                base = (h % 2) * 64
                nc.tensor.matmul(
                    stp[:, i, trim:],
                    kt[kc][base:base + HD, mt, kl * 128:(kl + 1) * 128],
                    qt[ci][base:base + HD, mt, trim:],
                    start=True, stop=True,
                )
            pt = pt_pool.tile([128, 2, CH], bf16, tag="pt")
            nc.scalar.activation(
                out=pt[:, :, trim:], in_=stp[:, :, trim:],
                func=mybir.ActivationFunctionType.Exp, scale=SCALE,
            )
            if tidx >= 0:
                nc.vector.tensor_mul(
                    pt[:, :, trim:trim + 128], pt[:, :, trim:trim + 128],
                    cmask[:, :, :],
                )
            pts_map[(ci, hp)].append((pt, trim))

        def av_seg(ci, hp, h, kj0, kj1, nkt):
            """AV accumulation for head h over key tiles [kj0, kj1)."""
            i = h % 2
            ot = ots_map[(ci, hp)][h]
            pts = pts_map[(ci, hp)]
            for kj in range(kj0, kj1):
                pt, trim = pts[kj]
                kc, kl = divmod(kj, KPC)
                nc.tensor.matmul(
                    ot[0:HD + 1, trim:],
                    vsb[kc][:, kl, h, :],
                    pt[:, i, trim:],
                    start=(kj == 0), stop=(kj == nkt - 1),
                )

        def epi(ci, hp, h):
            """1/l scale of the AV output into otn (no copies)."""
            base = (h % 2) * 64
            mt = hp
            ot = ots_map[(ci, hp)][h]
            l_sb = work.tile([1, CH], fp32, tag="l")
            nc.vector.tensor_copy(l_sb[:], ot[HD:HD + 1, :])
            rl = work.tile([1, CH], fp32, tag="rl")
            nc.vector.reciprocal_approx_fast(rl[:], l_sb[:])
            lb = work.tile([64, CH], fp32, tag="lb")
            nc.gpsimd.partition_broadcast(lb[:], rl[0:1, :])
            nc.vector.tensor_mul(
                otn[ci][base:base + HD, mt, :], ot[0:HD, :], lb[:],
            )

        yts = {}

        def proj_half(ci, half, last):
            if half == 0:
                yts[ci] = out_pool.tile([128, NT, CH], bf16, tag="yt", name="yt")
            for nt in range(half * NT2, (half + 1) * NT2):
                # last chunk: alternate PSUM between the mm and (now idle)
                # ot pools - 4 banks deep, so the matmuls aren't gated on
                # the evictions two tiles back.
                if last and nt % 2 == 1:
                    ps = ot_ps.tile([128, CH], fp32, tag="ot", name="otproj")
                else:
                    ps = mm_ps.tile([128, CH], fp32, tag="mm", name="mmps")
                for mt2 in range(MT):
                    nc.tensor.matmul(
                        ps[:],
                        wo[:, mt2, nt * 128:(nt + 1) * 128],
                        otn[ci][:, mt2, :],
                        start=(mt2 == 0), stop=(mt2 == MT - 1),
                    )
                # last chunk: split evictions between Scalar (exp is done
                # by now) and Vector, and DMA per nt-tile on alternating
                # rings, so eviction and writeback pipeline in the tail.
                if last and nt % 2 == 0:
                    nc.scalar.copy(yts[ci][:, nt, :], ps[:])
                else:
                    nc.vector.tensor_copy(yts[ci][:, nt, :], ps[:])
                if last:
                    eng = nc.scalar if nt % 2 == 0 else nc.sync
                    eng.dma_start(
                        out=yt_d[ci, half]
                        .rearrange("p (t s) -> p t s", s=CH)[:, nt - half * NT2, :],
                        in_=yts[ci][:, nt, :],
                    )
            if not last:
                nc.sync.dma_start(
                    out=yt_d[ci, half],
                    in_=yts[ci][:, half * NT2:(half + 1) * NT2, :]
                    .rearrange("p t s -> p (t s)"),
                )

        # ---- dependency-tagged two-queue scheduler ----------------------
        # `prio` holds ordered dataflow work (V slices a coming AV needs,
        # AV segments, epilogues, out-projections) - its internal order is
        # a correctness contract (ot-PSUM pool recycling).  `bulk` holds
        # position-free PE filler (k/q projection tiles, early V slices).
        # After each QK+exp unit ~fill_ns of work is popped, prio first,
        # so pair-boundary work interleaves into the next pair's exp
        # stream instead of lumping in front of it.
        prio = []
        bulk = []
        emitted = set()

        def emit_unit(q, idx):
            key, cost, fn = q.pop(idx)
            emitted.add(key)
            fn()
            return cost

        def force(key):
            if key in emitted:
                return
            for q in (prio, bulk):
                for idx, u in enumerate(q):
                    if u[0] == key:
                        emit_unit(q, idx)
                        return

        def pop_filler(budget):
            while budget > 0 and (prio or bulk):
                q = prio if prio else bulk
                budget -= emit_unit(q, 0)

        # pre-phase: kt0/qt0 (both only need wk/wq + xt0) so pair (0,0)
        # can start as soon as the first ~2MB of DMA lands.
        qk_mt(0, wk, kt, 0)
        qk_mt(0, wq, qt, 0)
        qk_mt(0, wk, kt, 1)
        qk_mt(0, wq, qt, 1)
        emitted |= {('k', 0, 0), ('k', 0, 1), ('q', 0, 0), ('q', 0, 1)}

        # unit costs at the PE's sustained 2.0 GHz (P0 power state)
        KQ_COST = 2100
        V_COST = 1100
        AV_COST = 1100
        PROJ_COST = 2100

        # bulk order = Vector-FIFO order of the ropes: q2 first (pair
        # (2,0) follows chunk 0), then k1/k2, k3/q3/q1; V slices last -
        # they are pulled into prio right before the AV that needs them.
        bulk.append((('q', 2, 0), KQ_COST, lambda: qk_mt(2, wq, qt, 0)))
        bulk.append((('q', 2, 1), KQ_COST, lambda: qk_mt(2, wq, qt, 1)))
        for c in (1, 2, 3):
            for mt in range(MT):
                bulk.append((('k', c, mt), KQ_COST,
                             lambda c=c, mt=mt: qk_mt(c, wk, kt, mt)))
        for c in (3, 1):
            for mt in range(MT):
                bulk.append((('q', c, mt), KQ_COST,
                             lambda c=c, mt=mt: qk_mt(c, wq, qt, mt)))
        for c in range(NCH):
            for sl in range(KPC):
                bulk.append((('v', c, sl), V_COST,
                             lambda c=c, sl=sl: v_sl(c, sl)))

        pair_order = [(0, 0), (2, 0), (2, 1), (3, 0),
                      (3, 1), (1, 0), (1, 1), (0, 1)]
        # per-pair PE filler budget (ns) after each QK+exp unit: ~balanced
        # against the 1.15us exp pace early, opened up late so the queues
        # drain before the exp stream ends.
        fill_ns = {0: 800, 2: 800, 3: 800, 1: 800}

        def keepwarm(n):
            # LDWEIGHTS-only: PE-array activity for the HAM clock monitor
            # without touching PSUM (all mid-kernel banks are live).
            for _ in range(n):
                nc.tensor.ldweights(warm_w[:])

        for pi, (ci, hp) in enumerate(pair_order):
            nkt = (ci + 1) * KPC
            pts_map[(ci, hp)] = []
            force(('q', ci, hp))
            for kj in range(nkt):
                kc = kj // KPC
                force(('k', kc, hp))
                # prefetch projections ~6 key-tiles ahead: the rope's
                # Vector latency (~3-5us behind the DVE queue) must clear
                # before the exp that needs it, or the ACT stream stalls.
                force(('k', min((kj + 6) // KPC, ci), hp))
                if kj == max(0, nkt - 6) and pi + 1 < len(pair_order):
                    ci2, hp2 = pair_order[pi + 1]
                    force(('q', ci2, hp2))
                    force(('k', 0, hp2))
                qk_kj(ci, hp, kj)
                pop_filler(fill_ns[ci])
                if pi >= len(pair_order) - 3:
                    # late pairs run ACT-paced with little PE work left;
                    # free matmuls keep the HAM clock at full rate.
                    keepwarm(3)
            # Queue this pair's AV + epilogues on prio.  Order contract:
            # 1) vsb[c<=ci] slices must precede the AV segments; 2) the
            # ot PSUM pool holds one pair (2 tiles), so pair p's AV must
            # be emitted after pair p-1's epilogues - guaranteed because
            # av/epi units only ever enter prio in pair order and prio is
            # FIFO.
            for c in range(ci + 1):
                for sl in range(KPC):
                    for idx, u in enumerate(bulk):
                        if u[0] == ('v', c, sl):
                            prio.append(bulk.pop(idx))
                            break
            ots_map[(ci, hp)] = {
                h: ot_ps.tile([128, CH], fp32, tag="ot", name=f"ot_{h}")
                for h in (2 * hp, 2 * hp + 1)
            }
            for h in (2 * hp, 2 * hp + 1):
                for kj0 in range(0, nkt, KPC):
                    kj1 = min(kj0 + KPC, nkt)
                    prio.append(
                        ((('av', ci, hp, h, kj0)), AV_COST,
                         lambda ci=ci, hp=hp, h=h, kj0=kj0, kj1=kj1, nkt=nkt:
                         av_seg(ci, hp, h, kj0, kj1, nkt)))
                prio.append(((('epi', ci, hp, h)), 100,
                             lambda ci=ci, hp=hp, h=h: epi(ci, hp, h)))
            # queue the chunk's out-projection once both its pairs' epis
            # are queued.
            both_done = all((ci, h) in ots_map for h in range(HP))
            if both_done and ('proj', ci, 0) not in emitted:
                last = pi == len(pair_order) - 1
                prio.append((('proj', ci, 0), PROJ_COST,
                             lambda ci=ci, last=last: proj_half(ci, 0, last)))
                prio.append((('proj', ci, 1), PROJ_COST,
                             lambda ci=ci, last=last: proj_half(ci, 1, last)))

        # drain remaining units, keeping the HAM clock warm between the
        # tail's dependency-chained bursts.
        while prio or bulk:
            q = prio if prio else bulk
            emit_unit(q, 0)
            keepwarm(4)


_CACHE = {}


def _get_nc(S, D, HC):
    key = (S, D, HC)
    if key not in _CACHE:
        nc = bacc.Bacc(None, target_bir_lowering=False)
        build_core(nc, S, D, HC)
        nc.compile()
        _CACHE[key] = nc
    return _CACHE[key]


def make_in_maps(x, rope_cos, rope_sin, W_qkv, W_out, n_cores=8):
    B, S, D = x.shape
    H = 16
    groups = n_cores // B          # head groups per batch
    HC = H // groups               # heads per core
    M = HC * HD
    MT = M // 128
    DT, NCH = D // 128, S // CH
    perm = rope_perm()
    bf16 = ml_dtypes.bfloat16
    cs, sn = rope_tables(np.asarray(rope_cos), np.asarray(rope_sin), S)
    # chunk-major, in the kernel's DMA-arrival order [0, 2, 1, 3]; each
    # chunk block is [cos; sin] ([128, 2, CH]).
    csn = np.concatenate(
        [np.stack([cs[:, c * CH:(c + 1) * CH], sn[:, c * CH:(c + 1) * CH]],
                  axis=1) for c in (0, 2, 1, 3)],
        axis=1).reshape(128, 2 * NCH * CH)
    in_maps = []
    xtb_cache = {}
    for c in range(n_cores):
        b, g = divmod(c, groups)
        heads = np.arange(g * HC, (g + 1) * HC)
        qcols = np.concatenate([h * HD + perm for h in heads])
        vcols = np.concatenate([2 * D + h * HD + np.arange(HD) for h in heads])
        if b not in xtb_cache:
            xtb_cache[b] = np.ascontiguousarray(
                np.asarray(x[b]).T.reshape(DT, 128, NCH, CH)
                .transpose(2, 1, 0, 3).reshape(NCH, 128, DT * CH)
            ).astype(bf16)

        def wfmt(wcols):
            return np.ascontiguousarray(
                wcols.reshape(DT, 128, M).transpose(1, 0, 2).reshape(128, DT * M)
            ).astype(bf16)

        wo_np = np.ascontiguousarray(
            W_out[g * M:(g + 1) * M, :].reshape(MT, 128, D)
            .transpose(1, 0, 2).reshape(128, MT * D)).astype(bf16)
        in_maps.append({
            "xt": xtb_cache[b],
            "wq": wfmt(W_qkv[:, qcols]),
            "wk": wfmt(W_qkv[:, D + qcols]),
            "wv": wfmt(W_qkv[:, vcols]),
            "wo": wo_np,
            "csn": np.ascontiguousarray(csn).astype(bf16),
        })
    return in_maps


def unshard_out(res, B, S, D, n_cores=8):
    NCH, NT = S // CH, D // 128
    NT2 = NT // 2
    out = np.zeros((B, S, D), np.float32)
    for c in range(n_cores):
        yt = res.results[c]["yt"].astype(np.float32)  # [NCH, 2, 128, NT2*CH]
        ytf = (yt.reshape(NCH, 2, 128, NT2, CH)
               .transpose(1, 3, 2, 0, 4).reshape(D, S))
        out[c // (n_cores // B)] += ytf.T
    return out


def kernel(x, rope_cos, rope_sin, W_qkv, W_out):
    x = np.asarray(x)
    W_qkv = np.asarray(W_qkv)
    W_out = np.asarray(W_out)
    B, S, D = x.shape
    n_cores = 8
    HC = 16 // (n_cores // B)
    in_maps = make_in_maps(x, rope_cos, rope_sin, W_qkv, W_out, n_cores)
    nc = _get_nc(S, D, HC)
    res = run_bass_kernel_spmd(nc, in_maps, list(range(n_cores)))
    return unshard_out(res, B, S, D, n_cores)


# revision 36
# speedup vs baseline: 1.0103x; 1.0103x over previous
"""Causal self-attention (B=2, S=2048, D=1024, H=16) on 8 TRN2 NeuronCores.

Sharding: core c -> batch b = c//4, head group g = c%4 (4 heads each).
Each core computes the qkv projection for its heads, RoPE, causal flash
attention, and a partial out-projection (row-parallel); the host sums the
4 partials per batch.

Layout strategy (everything "transposed", seq on the free axis):
  xt  = x[b]^T                  [D, S]   bf16 (host-prepped)
  Qt/Kt[m, s] per head          computed as  W[:,m]^T @ xt  (lhsT = W slice)
  V natural [s, m]              computed as  xt_tile^T @ Wv
  St[k, q]  = Kt_tile^T @ Qt    -> exp -> causal mask -> Pt (bf16)
  Ot'[m+1, q] = [V|1]^T @ Pt    (row m==HD is the softmax denominator l[q])
  y^T[n, s] = Wo[:,n]^T @ (Ot/l)  accumulated over m tiles; host sums cores.

RoPE: interleaved rotate made partition-aligned by permuting W columns on
the host; on-device combine = 2 muls + stream_shuffle + add (Vector).

Perf structure (v3, 168us vs 185us baseline):
 - All inputs land via 9 large DMAs on the sync HWDGE ring instead of ~70
   small ones (each dma_start costs ~615ns of serialized issue), so the
   first matmul starts ~3us in and the HAM clock-gate (which halves the PE
   clock after ~3.4us of low activity) never re-throttles mid-kernel.
 - 10 dummy matmuls at t=0 warm the PE clock while the DMAs stream; 8 more
   before the last out-projection cover the exp-bound tail.
 - Work is emitted chunk-by-chunk with attention of chunk ci interleaved
   with the qkv of chunk ci+2 and the out-projection of ci-1 (PE filler
   for the ACT-exp-paced softmax, ~1us per key-tile pair). Emission order
   per engine follows dataflow order pair-by-pair: reordering across pairs
   creates multi-us cross-FIFO convoys (engines execute in FIFO order).
 - Output is evicted to bf16 and written with 2 large DMAs per chunk.
 - RoPE add on GpSimd; l broadcast on GpSimd; evictions on Vector (last
   chunk on Scalar, which is idle once exp finishes).
"""

from contextlib import ExitStack

import numpy as np
import ml_dtypes

import concourse.bass as bass
import concourse.tile as tile
import concourse.mybir as mybir
from concourse import bacc
from concourse.bass_utils import run_bass_kernel_spmd

HD = 64          # head dim
CH = 512         # seq chunk (one PSUM bank of fp32)
_SHUF = [(i + 16) % 32 for i in range(32)]  # swap 16-halves in each quadrant


def rope_perm():
    """Within-head output-column permutation: local row r <- reference col."""
    perm = np.zeros(HD, dtype=np.int64)
    for r in range(HD):
        q, pos = divmod(r, 32)
        x2 = pos >= 16
        f = q * 16 + (pos % 16)
        perm[r] = 2 * f + (1 if x2 else 0)
    return perm


def rope_tables(rope_cos, rope_sin, S):
    """cos/sin tables [128, S] fp32 aligned with the permuted Qt/Kt rows."""
    cs = np.zeros((128, S), np.float32)
    sn = np.zeros((128, S), np.float32)
    for r in range(128):
        rr = r % HD
        q, pos = divmod(rr, 32)
        x2 = pos >= 16
        f = q * 16 + (pos % 16)
        cs[r] = rope_cos[:S, f]
        sn[r] = rope_sin[:S, f] * (-1.0 if x2 else 1.0)
    return cs, sn


def merge_prop(a, b):
    """Merge two unit lists, advancing each proportionally to its length."""
    out = []
    ia = ib = 0
    while ia < len(a) or ib < len(b):
        fa = ia / len(a) if a else 1.0
        fb = ib / len(b) if b else 1.0
        if ia < len(a) and (ib >= len(b) or fa <= fb):
            out.append(a[ia]); ia += 1
        else:
            out.append(b[ib]); ib += 1
    return out


def build_core(nc, S, D, HC):
    """Emit the per-core kernel IR. HC = heads on this core."""
    DT = D // 128           # contraction tiles over model dim
    M = HC * HD             # local qkv width
    MT = M // 128           # m tiles
    NCH = S // CH           # seq chunks
    KPC = CH // 128         # key tiles per chunk
    NT = D // 128           # out-proj n tiles
    NT2 = NT // 2
    HP = HC // 2            # head pairs
    fp32, bf16 = mybir.dt.float32, mybir.dt.bfloat16
    SCALE = float(HD) ** -0.5

    xt_d = nc.declare_dram_parameter("xt", [NCH, 128, DT * CH], bf16, isOutput=False)
    wq_d = nc.declare_dram_parameter("wq", [128, DT * M], bf16, isOutput=False)
    wk_d = nc.declare_dram_parameter("wk", [128, DT * M], bf16, isOutput=False)
    wv_d = nc.declare_dram_parameter("wv", [128, DT * M], bf16, isOutput=False)
    wo_d = nc.declare_dram_parameter("wo", [128, MT * D], bf16, isOutput=False)
    csn_d = nc.declare_dram_parameter("csn", [128, 2 * NCH * CH], bf16, isOutput=False)
    yt_d = nc.declare_dram_parameter("yt", [NCH, 2, 128, NT2 * CH], bf16, isOutput=True)

    with tile.TileContext(nc) as tc, ExitStack() as ctx:
        persist = ctx.enter_context(tc.tile_pool(name="persist", bufs=1))
        mm_ps = ctx.enter_context(tc.tile_pool(name="mm_ps", bufs=2, space="PSUM"))
        st_ps = ctx.enter_context(tc.tile_pool(name="st_ps", bufs=2, space="PSUM"))
        ot_ps = ctx.enter_context(tc.tile_pool(name="ot_ps", bufs=2, space="PSUM"))
        work = ctx.enter_context(tc.tile_pool(name="work", bufs=3))
        pt_pool = ctx.enter_context(tc.tile_pool(name="ptp", bufs=20))
        out_pool = ctx.enter_context(tc.tile_pool(name="outp", bufs=2))

        # ---- persistent tiles -------------------------------------------
        xt = [persist.tile([128, DT, CH], bf16, name=f"xt_{c}") for c in range(NCH)]
        wq = persist.tile([128, DT, M], bf16)
        wk = persist.tile([128, DT, M], bf16)
        wv = persist.tile([128, DT, M], bf16)
        wo = persist.tile([128, MT, D], bf16)
        csn = persist.tile([128, 2, NCH, CH], bf16)
        qt = [persist.tile([128, MT, CH], bf16, name=f"qt_{c}") for c in range(NCH)]
        kt = [persist.tile([128, MT, CH], bf16, name=f"kt_{c}") for c in range(NCH)]
        vsb = [persist.tile([128, KPC, HC, HD + 1], bf16, name=f"vsb_{c}")
               for c in range(NCH)]
        otn = [persist.tile([128, MT, CH], bf16, name=f"otn_{c}") for c in range(NCH)]
        cmask = persist.tile([128, KPC, 2, CH], bf16)

        # ---- input DMAs: few and large, split across the two HWDGE rings
        nc.sync.dma_start(out=wq.rearrange("p t m -> p (t m)"), in_=wq_d[:, :])
        nc.sync.dma_start(
            out=xt[0].rearrange("p t s -> p (t s)"), in_=xt_d[0])
        nc.sync.dma_start(
            out=csn.rearrange("p i c s -> p (i c s)"), in_=csn_d[:, :])
        nc.sync.dma_start(out=wk.rearrange("p t m -> p (t m)"), in_=wk_d[:, :])
        nc.sync.dma_start(out=wv.rearrange("p t m -> p (t m)"), in_=wv_d[:, :])
        for c in range(1, NCH):
            nc.sync.dma_start(
                out=xt[c].rearrange("p t s -> p (t s)"), in_=xt_d[c])
        nc.sync.dma_start(out=wo.rearrange("p t n -> p (t n)"), in_=wo_d[:, :])

        # ---- PE warmup: HAM clock un-throttle while DMAs stream ---------
        # ~128 small matmuls keep the PE busy (and at full clock) until the
        # first real matmul's inputs land (~16us); a dummy exp preloads the
        # ACT table set (~2.7us) off the first real exp's critical path.
        warm_w = persist.tile([128, 128], bf16)
        warm_x = persist.tile([128, CH], bf16)
        nc.vector.memset(warm_w[:], 0.0)
        nc.vector.memset(warm_x[:], 0.0)
        warm_act = work.tile([128, 16], fp32, tag="wact")
        nc.scalar.activation(out=warm_act[:], in_=warm_w[:, 0:16],
                             func=mybir.ActivationFunctionType.Exp)
        warm_ps = mm_ps.tile([128, CH], fp32, tag="mm", name="mmps")
        for _ in range(128):
            nc.tensor.matmul(warm_ps[:, 0:128], warm_w[:], warm_w[:],
                             start=True, stop=True)

        # causal masks for diagonal tiles (both head slots): keep
        # j - p - 128*t >= 0. Built on GpSimd while it is otherwise idle.
        nc.gpsimd.memset(cmask[:], 1.0)
        for t in range(KPC):
            for i in range(2):
                nc.gpsimd.affine_select(
                    out=cmask[:, t, i, :], in_=cmask[:, t, i, :],
                    compare_op=mybir.AluOpType.is_ge, fill=0.0,
                    base=-128 * t, pattern=[[1, CH]], channel_multiplier=-1,
                )
        for c in range(NCH):
            nc.vector.memset(vsb[c][:, :, :, HD:HD + 1], 1.0)

        # ---- emission units ---------------------------------------------
        def qk_part(ci, wt, dst):
            for mt in range(MT):
                ps = mm_ps.tile([128, CH], fp32, tag="mm", name="mmps")
                for dt in range(DT):
                    nc.tensor.matmul(
                        ps[:],
                        wt[:, dt, mt * 128:(mt + 1) * 128],
                        xt[ci][:, dt, :],
                        start=(dt == 0), stop=(dt == DT - 1),
                    )
                p1 = work.tile([128, CH], fp32, tag="p1")
                p2 = work.tile([128, CH], fp32, tag="p2")
                p2s = work.tile([128, CH], fp32, tag="p2s")
                nc.vector.tensor_mul(p1[:], ps[:], csn[:, 0, ci, :])
                nc.vector.tensor_mul(p2[:], ps[:], csn[:, 1, ci, :])
                nc.vector.stream_shuffle(p2s[:], p2[:], mask=_SHUF)
                nc.vector.tensor_add(dst[ci][:, mt, :], p1[:], p2s[:])

        def v_part(ci):
            for sl in range(KPC):
                ps = mm_ps.tile([128, M], fp32, tag="mm", name="vps")
                for dt in range(DT):
                    nc.tensor.matmul(
                        ps[:],
                        xt[ci][:, dt, sl * 128:(sl + 1) * 128],
                        wv[:, dt, :],
                        start=(dt == 0), stop=(dt == DT - 1),
                    )
                nc.vector.tensor_copy(
                    vsb[ci][:, sl, :, 0:HD],
                    ps.rearrange("p (h d) -> p h d", h=HC),
                )

        def attn_part(ci, hp):
            nkt = (ci + 1) * KPC
            heads = (2 * hp, 2 * hp + 1)
            mt = hp
            ots = {}
            for h in heads:
                ots[h] = ot_ps.tile([128, CH], fp32, tag="ot", name=f"ot_{h}")
            pts = []
            for kj in range(nkt):
                tidx = kj - ci * KPC
                trim = max(0, tidx) * 128
                kc, kl = divmod(kj, KPC)
                stp = st_ps.tile([128, 2, CH], fp32, tag="st")
                for i, h in enumerate(heads):
                    base = (h % 2) * 64
                    nc.tensor.matmul(
                        stp[:, i, trim:],
                        kt[kc][base:base + HD, mt, kl * 128:(kl + 1) * 128],
                        qt[ci][base:base + HD, mt, trim:],
                        start=True, stop=True,
                    )
                pt = pt_pool.tile([128, 2, CH], bf16, tag="pt")
                nc.scalar.activation(
                    out=pt[:, :, trim:], in_=stp[:, :, trim:],
                    func=mybir.ActivationFunctionType.Exp, scale=SCALE,
                )
                if tidx >= 0:
                    for i in range(2):
                        nc.vector.tensor_mul(
                            pt[:, i, trim:], pt[:, i, trim:],
                            cmask[:, tidx, i, trim:],
                        )
                pts.append((pt, trim))
            for i, h in enumerate(heads):
                for kj in range(nkt):
                    pt, trim = pts[kj]
                    kc, kl = divmod(kj, KPC)
                    nc.tensor.matmul(
                        ots[h][0:HD + 1, trim:],
                        vsb[kc][:, kl, h, :],
                        pt[:, i, trim:],
                        start=(kj == 0), stop=(kj == nkt - 1),
                    )
            for h in heads:
                base = (h % 2) * 64
                ot = ots[h]
                osb = work.tile([HD, CH], fp32, tag="osb")
                nc.vector.tensor_copy(osb[:], ot[0:HD, :])
                l_sb = work.tile([1, CH], fp32, tag="l")
                nc.vector.tensor_copy(l_sb[:], ot[HD:HD + 1, :])
                lb = work.tile([64, CH], fp32, tag="lb")
                nc.gpsimd.partition_broadcast(lb[:], l_sb[0:1, :])
                rl = work.tile([64, CH], fp32, tag="rl")
                nc.vector.reciprocal_approx_fast(rl[:], lb[:])
                nc.vector.tensor_mul(
                    otn[ci][base:base + HD, mt, :], osb[:], rl[:],
                )

        yts = {}

        def proj_part(ci, half):
            if half == 0:
                yts[ci] = out_pool.tile([128, NT, CH], bf16, tag="yt", name="yt")
            for nt in range(half * NT2, (half + 1) * NT2):
                ps = mm_ps.tile([128, CH], fp32, tag="mm", name="mmps")
                for mt2 in range(MT):
                    nc.tensor.matmul(
                        ps[:],
                        wo[:, mt2, nt * 128:(nt + 1) * 128],
                        otn[ci][:, mt2, :],
                        start=(mt2 == 0), stop=(mt2 == MT - 1),
                    )
                if ci == NCH - 1:
                    nc.scalar.copy(yts[ci][:, nt, :], ps[:])
                else:
                    nc.vector.tensor_copy(yts[ci][:, nt, :], ps[:])
            nc.sync.dma_start(
                out=yt_d[ci, half],
                in_=yts[ci][:, half * NT2:(half + 1) * NT2, :]
                .rearrange("p t s -> p (t s)"),
            )

        from itertools import zip_longest

        def interleave(*streams):
            for group in zip_longest(*streams):
                for fn in group:
                    if fn is not None:
                        fn()

        def qkv_units(ci):
            return [lambda: qk_part(ci, wq, qt),
                    lambda: qk_part(ci, wk, kt),
                    lambda: v_part(ci)]

        def attn_units(ci):
            return [(lambda hp=hp: attn_part(ci, hp)) for hp in range(HP)]

        def proj_units(ci):
            return [lambda: proj_part(ci, 0), lambda: proj_part(ci, 1)]

        interleave(qkv_units(0))
        if NCH == 1:
            interleave(attn_units(0))
            interleave(proj_units(0))
        else:
            interleave(qkv_units(1))
            for ci in range(NCH - 2):
                streams = [attn_units(ci), qkv_units(ci + 2)]
                if ci >= 1:
                    streams.append(proj_units(ci - 1))
                interleave(*streams)
            interleave(attn_units(NCH - 2),
                       proj_units(NCH - 3) if NCH >= 3 else [])
            interleave(attn_units(NCH - 1),
                       proj_units(NCH - 2) if NCH >= 2 else [])
            ka_ps = mm_ps.tile([128, CH], fp32, tag="mm", name="ka_ps")
            for _ in range(8):
                nc.tensor.matmul(ka_ps[:], warm_w[:],
                                 otn[NCH - 1][:, 0, :], start=True, stop=True)
            interleave(proj_units(NCH - 1))




_CACHE = {}


def _get_nc(S, D, HC):
    key = (S, D, HC)
    if key not in _CACHE:
        nc = bacc.Bacc(None, target_bir_lowering=False)
        build_core(nc, S, D, HC)
        nc.compile()
        _CACHE[key] = nc
    return _CACHE[key]


def make_in_maps(x, rope_cos, rope_sin, W_qkv, W_out, n_cores=8):
    B, S, D = x.shape
    H = 16
    groups = n_cores // B          # head groups per batch
    HC = H // groups               # heads per core
    M = HC * HD
    MT = M // 128
    DT, NCH = D // 128, S // CH
    perm = rope_perm()
    bf16 = ml_dtypes.bfloat16
    cs, sn = rope_tables(np.asarray(rope_cos), np.asarray(rope_sin), S)
    csn = np.stack([cs.reshape(128, NCH * CH), sn.reshape(128, NCH * CH)],
                   axis=1).reshape(128, 2 * NCH * CH)
    in_maps = []
    xtb_cache = {}
    for c in range(n_cores):
        b, g = divmod(c, groups)
        heads = np.arange(g * HC, (g + 1) * HC)
        qcols = np.concatenate([h * HD + perm for h in heads])
        vcols = np.concatenate([2 * D + h * HD + np.arange(HD) for h in heads])
        if b not in xtb_cache:
            xtb_cache[b] = np.ascontiguousarray(
                np.asarray(x[b]).T.reshape(DT, 128, NCH, CH)
                .transpose(2, 1, 0, 3).reshape(NCH, 128, DT * CH)
            ).astype(bf16)

        def wfmt(wcols):
            return np.ascontiguousarray(
                wcols.reshape(DT, 128, M).transpose(1, 0, 2).reshape(128, DT * M)
            ).astype(bf16)

        wo_np = np.ascontiguousarray(
            W_out[g * M:(g + 1) * M, :].reshape(MT, 128, D)
            .transpose(1, 0, 2).reshape(128, MT * D)).astype(bf16)
        in_maps.append({
            "xt": xtb_cache[b],
            "wq": wfmt(W_qkv[:, qcols]),
            "wk": wfmt(W_qkv[:, D + qcols]),
            "wv": wfmt(W_qkv[:, vcols]),
            "wo": wo_np,
            "csn": np.ascontiguousarray(csn).astype(bf16),
        })
    return in_maps


def unshard_out(res, B, S, D, n_cores=8):
    NCH, NT = S // CH, D // 128
    NT2 = NT // 2
    out = np.zeros((B, S, D), np.float32)
    for c in range(n_cores):
        yt = res.results[c]["yt"].astype(np.float32)  # [NCH, 2, 128, NT2*CH]
        ytf = (yt.reshape(NCH, 2, 128, NT2, CH)
               .transpose(1, 3, 2, 0, 4).reshape(D, S))
        out[c // (n_cores // B)] += ytf.T
    return out


def kernel(x, rope_cos, rope_sin, W_qkv, W_out):
    x = np.asarray(x)
    W_qkv = np.asarray(W_qkv)
    W_out = np.asarray(W_out)
    B, S, D = x.shape
    n_cores = 8
    HC = 16 // (n_cores // B)
    in_maps = make_in_maps(x, rope_cos, rope_sin, W_qkv, W_out, n_cores)
    nc = _get_nc(S, D, HC)
    res = run_bass_kernel_spmd(nc, in_maps, list(range(n_cores)))
    return unshard_out(res, B, S, D, n_cores)



# revision 40
# speedup vs baseline: 1.0561x; 1.0453x over previous
"""Causal self-attention (B=2, S=2048, D=1024, H=16) on 8 TRN2 NeuronCores.

Sharding: core c -> batch b = c//4, head group g = c%4 (4 heads each).
Each core computes the qkv projection for its heads, RoPE, causal flash
attention, and a partial out-projection (row-parallel); the host sums the
4 partials per batch.

Layout strategy (everything "transposed", seq on the free axis):
  xt  = x[b]^T                  [D, S]   bf16 (host-prepped)
  Qt/Kt[m, s] per head          computed as  W[:,m]^T @ xt  (lhsT = W slice)
  V natural [s, m]              computed as  xt_tile^T @ Wv
  St[k, q]  = Kt_tile^T @ Qt    -> exp -> causal mask -> Pt (bf16)
  Ot'[m+1, q] = [V|1]^T @ Pt    (row m==HD is the softmax denominator l[q])
  y^T[n, s] = Wo[:,n]^T @ (Ot/l)  accumulated over m tiles; host sums cores.

RoPE: interleaved rotate made partition-aligned by permuting W columns on
the host; on-device combine = 2 muls + stream_shuffle + add (Vector).

Perf structure (v3, 168us vs 185us baseline):
 - All inputs land via 9 large DMAs on the sync HWDGE ring instead of ~70
   small ones (each dma_start costs ~615ns of serialized issue), so the
   first matmul starts ~3us in and the HAM clock-gate (which halves the PE
   clock after ~3.4us of low activity) never re-throttles mid-kernel.
 - 10 dummy matmuls at t=0 warm the PE clock while the DMAs stream; 8 more
   before the last out-projection cover the exp-bound tail.
 - Work is emitted chunk-by-chunk with attention of chunk ci interleaved
   with the qkv of chunk ci+2 and the out-projection of ci-1 (PE filler
   for the ACT-exp-paced softmax, ~1us per key-tile pair). Emission order
   per engine follows dataflow order pair-by-pair: reordering across pairs
   creates multi-us cross-FIFO convoys (engines execute in FIFO order).
 - Output is evicted to bf16 and written with 2 large DMAs per chunk.
 - RoPE add on GpSimd; l broadcast on GpSimd; evictions on Vector (last
   chunk on Scalar, which is idle once exp finishes).
"""

from contextlib import ExitStack

import numpy as np
import ml_dtypes

import concourse.bass as bass
import concourse.tile as tile
import concourse.mybir as mybir
from concourse import bacc
from concourse.bass_utils import run_bass_kernel_spmd

HD = 64          # head dim
CH = 512         # seq chunk (one PSUM bank of fp32)
_SHUF = [(i + 16) % 32 for i in range(32)]  # swap 16-halves in each quadrant


def rope_perm():
    """Within-head output-column permutation: local row r <- reference col."""
    perm = np.zeros(HD, dtype=np.int64)
    for r in range(HD):
        q, pos = divmod(r, 32)
        x2 = pos >= 16
        f = q * 16 + (pos % 16)
        perm[r] = 2 * f + (1 if x2 else 0)
    return perm


def rope_tables(rope_cos, rope_sin, S):
    """cos/sin tables [128, S] fp32 aligned with the permuted Qt/Kt rows."""
    cs = np.zeros((128, S), np.float32)
    sn = np.zeros((128, S), np.float32)
    for r in range(128):
        rr = r % HD
        q, pos = divmod(rr, 32)
        x2 = pos >= 16
        f = q * 16 + (pos % 16)
        cs[r] = rope_cos[:S, f]
        sn[r] = rope_sin[:S, f] * (-1.0 if x2 else 1.0)
    return cs, sn


def merge_prop(a, b):
    """Merge two unit lists, advancing each proportionally to its length."""
    out = []
    ia = ib = 0
    while ia < len(a) or ib < len(b):
        fa = ia / len(a) if a else 1.0
        fb = ib / len(b) if b else 1.0
        if ia < len(a) and (ib >= len(b) or fa <= fb):
            out.append(a[ia]); ia += 1
        else:
            out.append(b[ib]); ib += 1
    return out


def build_core(nc, S, D, HC):
    """Emit the per-core kernel IR. HC = heads on this core."""
    DT = D // 128           # contraction tiles over model dim
    M = HC * HD             # local qkv width
    MT = M // 128           # m tiles
    NCH = S // CH           # seq chunks
    KPC = CH // 128         # key tiles per chunk
    NT = D // 128           # out-proj n tiles
    NT2 = NT // 2
    HP = HC // 2            # head pairs
    fp32, bf16 = mybir.dt.float32, mybir.dt.bfloat16
    SCALE = float(HD) ** -0.5

    xt_d = nc.declare_dram_parameter("xt", [NCH, 128, DT * CH], bf16, isOutput=False)
    wq_d = nc.declare_dram_parameter("wq", [128, DT * M], bf16, isOutput=False)
    wk_d = nc.declare_dram_parameter("wk", [128, DT * M], bf16, isOutput=False)
    wv_d = nc.declare_dram_parameter("wv", [128, DT * M], bf16, isOutput=False)
    wo_d = nc.declare_dram_parameter("wo", [128, MT * D], bf16, isOutput=False)
    csn_d = nc.declare_dram_parameter("csn", [128, 2 * NCH * CH], bf16, isOutput=False)
    yt_d = nc.declare_dram_parameter("yt", [NCH, 2, 128, NT2 * CH], bf16, isOutput=True)

    with tile.TileContext(nc) as tc, ExitStack() as ctx:
        persist = ctx.enter_context(tc.tile_pool(name="persist", bufs=1))
        mm_ps = ctx.enter_context(tc.tile_pool(name="mm_ps", bufs=2, space="PSUM"))
        st_ps = ctx.enter_context(tc.tile_pool(name="st_ps", bufs=2, space="PSUM"))
        ot_ps = ctx.enter_context(tc.tile_pool(name="ot_ps", bufs=2, space="PSUM"))
        work = ctx.enter_context(tc.tile_pool(name="work", bufs=3))
        pt_pool = ctx.enter_context(tc.tile_pool(name="ptp", bufs=20))
        out_pool = ctx.enter_context(tc.tile_pool(name="outp", bufs=2))

        # ---- persistent tiles -------------------------------------------
        xt = [persist.tile([128, DT, CH], bf16, name=f"xt_{c}") for c in range(NCH)]
        wq = persist.tile([128, DT, M], bf16)
        wk = persist.tile([128, DT, M], bf16)
        wv = persist.tile([128, DT, M], bf16)
        wo = persist.tile([128, MT, D], bf16)
        csn = persist.tile([128, 2, NCH, CH], bf16)
        qt = [persist.tile([128, MT, CH], bf16, name=f"qt_{c}") for c in range(NCH)]
        kt = [persist.tile([128, MT, CH], bf16, name=f"kt_{c}") for c in range(NCH)]
        vsb = [persist.tile([128, KPC, HC, HD + 1], bf16, name=f"vsb_{c}")
               for c in range(NCH)]
        otn = [persist.tile([128, MT, CH], bf16, name=f"otn_{c}") for c in range(NCH)]
        cmask = persist.tile([128, 2, 128], bf16)

        # ---- input DMAs: few and large, split across the two HWDGE rings
        nc.sync.dma_start(out=wq.rearrange("p t m -> p (t m)"), in_=wq_d[:, :])
        nc.sync.dma_start(
            out=xt[0].rearrange("p t s -> p (t s)"), in_=xt_d[0])
        nc.sync.dma_start(
            out=csn.rearrange("p i c s -> p (i c s)"), in_=csn_d[:, :])
        nc.sync.dma_start(out=wk.rearrange("p t m -> p (t m)"), in_=wk_d[:, :])
        nc.sync.dma_start(out=wv.rearrange("p t m -> p (t m)"), in_=wv_d[:, :])
        for c in range(1, NCH):
            nc.sync.dma_start(
                out=xt[c].rearrange("p t s -> p (t s)"), in_=xt_d[c])
        nc.sync.dma_start(out=wo.rearrange("p t n -> p (t n)"), in_=wo_d[:, :])

        # ---- PE warmup: HAM clock un-throttle while DMAs stream ---------
        # ~128 small matmuls keep the PE busy (and at full clock) until the
        # first real matmul's inputs land (~16us); a dummy exp preloads the
        # ACT table set (~2.7us) off the first real exp's critical path.
        warm_w = persist.tile([128, 128], bf16)
        warm_x = persist.tile([128, CH], bf16)
        nc.vector.memset(warm_w[:], 0.0)
        nc.vector.memset(warm_x[:], 0.0)
        warm_act = work.tile([128, 16], fp32, tag="wact")
        nc.scalar.activation(out=warm_act[:], in_=warm_w[:, 0:16],
                             func=mybir.ActivationFunctionType.Exp)
        warm_ps = mm_ps.tile([128, CH], fp32, tag="mm", name="mmps")
        for _ in range(128):
            nc.tensor.matmul(warm_ps[:, 0:128], warm_w[:], warm_w[:],
                             start=True, stop=True)

        # causal mask for the 128-col diagonal window (both head slots):
        # keep j - p >= 0 (query-local j, key-local p) - identical for
        # every diagonal tile; columns past the window are never masked.
        # Built on GpSimd while it is otherwise idle.
        nc.gpsimd.memset(cmask[:], 1.0)
        for i in range(2):
            nc.gpsimd.affine_select(
                out=cmask[:, i, :], in_=cmask[:, i, :],
                compare_op=mybir.AluOpType.is_ge, fill=0.0,
                base=0, pattern=[[1, 128]], channel_multiplier=-1,
            )
        for c in range(NCH):
            nc.vector.memset(vsb[c][:, :, :, HD:HD + 1], 1.0)

        # ---- emission units ---------------------------------------------
        def qk_part(ci, wt, dst):
            for mt in range(MT):
                ps = mm_ps.tile([128, CH], fp32, tag="mm", name="mmps")
                for dt in range(DT):
                    nc.tensor.matmul(
                        ps[:],
                        wt[:, dt, mt * 128:(mt + 1) * 128],
                        xt[ci][:, dt, :],
                        start=(dt == 0), stop=(dt == DT - 1),
                    )
                p1 = work.tile([128, CH], fp32, tag="p1")
                p2 = work.tile([128, CH], fp32, tag="p2")
                p2s = work.tile([128, CH], fp32, tag="p2s")
                nc.vector.tensor_mul(p1[:], ps[:], csn[:, 0, ci, :])
                nc.vector.tensor_mul(p2[:], ps[:], csn[:, 1, ci, :])
                nc.vector.stream_shuffle(p2s[:], p2[:], mask=_SHUF)
                nc.vector.tensor_add(dst[ci][:, mt, :], p1[:], p2s[:])

        def v_part(ci):
            for sl in range(KPC):
                ps = mm_ps.tile([128, M], fp32, tag="mm", name="vps")
                for dt in range(DT):
                    nc.tensor.matmul(
                        ps[:],
                        xt[ci][:, dt, sl * 128:(sl + 1) * 128],
                        wv[:, dt, :],
                        start=(dt == 0), stop=(dt == DT - 1),
                    )
                nc.vector.tensor_copy(
                    vsb[ci][:, sl, :, 0:HD],
                    ps.rearrange("p (h d) -> p h d", h=HC),
                )

        def attn_part(ci, hp):
            nkt = (ci + 1) * KPC
            heads = (2 * hp, 2 * hp + 1)
            mt = hp
            ots = {}
            for h in heads:
                ots[h] = ot_ps.tile([128, CH], fp32, tag="ot", name=f"ot_{h}")
            pts = []
            for kj in range(nkt):
                tidx = kj - ci * KPC
                trim = max(0, tidx) * 128
                kc, kl = divmod(kj, KPC)
                stp = st_ps.tile([128, 2, CH], fp32, tag="st")
                for i, h in enumerate(heads):
                    base = (h % 2) * 64
                    nc.tensor.matmul(
                        stp[:, i, trim:],
                        kt[kc][base:base + HD, mt, kl * 128:(kl + 1) * 128],
                        qt[ci][base:base + HD, mt, trim:],
                        start=True, stop=True,
                    )
                pt = pt_pool.tile([128, 2, CH], bf16, tag="pt")
                nc.scalar.activation(
                    out=pt[:, :, trim:], in_=stp[:, :, trim:],
                    func=mybir.ActivationFunctionType.Exp, scale=SCALE,
                )
                if tidx >= 0:
                    nc.vector.tensor_mul(
                        pt[:, :, trim:trim + 128], pt[:, :, trim:trim + 128],
                        cmask[:, :, :],
                    )
                pts.append((pt, trim))
            for i, h in enumerate(heads):
                for kj in range(nkt):
                    pt, trim = pts[kj]
                    kc, kl = divmod(kj, KPC)
                    nc.tensor.matmul(
                        ots[h][0:HD + 1, trim:],
                        vsb[kc][:, kl, h, :],
                        pt[:, i, trim:],
                        start=(kj == 0), stop=(kj == nkt - 1),
                    )
            for h in heads:
                base = (h % 2) * 64
                ot = ots[h]
                l_sb = work.tile([1, CH], fp32, tag="l")
                nc.vector.tensor_copy(l_sb[:], ot[HD:HD + 1, :])
                rl = work.tile([1, CH], fp32, tag="rl")
                nc.vector.reciprocal_approx_fast(rl[:], l_sb[:])
                lb = work.tile([64, CH], fp32, tag="lb")
                nc.gpsimd.partition_broadcast(lb[:], rl[0:1, :])
                nc.vector.tensor_mul(
                    otn[ci][base:base + HD, mt, :], ot[0:HD, :], lb[:],
                )

        yts = {}

        def proj_part(ci, half):
            if half == 0:
                yts[ci] = out_pool.tile([128, NT, CH], bf16, tag="yt", name="yt")
            for nt in range(half * NT2, (half + 1) * NT2):
                ps = mm_ps.tile([128, CH], fp32, tag="mm", name="mmps")
                for mt2 in range(MT):
                    nc.tensor.matmul(
                        ps[:],
                        wo[:, mt2, nt * 128:(nt + 1) * 128],
                        otn[ci][:, mt2, :],
                        start=(mt2 == 0), stop=(mt2 == MT - 1),
                    )
                if ci == NCH - 1:
                    nc.scalar.copy(yts[ci][:, nt, :], ps[:])
                else:
                    nc.vector.tensor_copy(yts[ci][:, nt, :], ps[:])
            nc.sync.dma_start(
                out=yt_d[ci, half],
                in_=yts[ci][:, half * NT2:(half + 1) * NT2, :]
                .rearrange("p t s -> p (t s)"),
            )

        from itertools import zip_longest

        def interleave(*streams):
            for group in zip_longest(*streams):
                for fn in group:
                    if fn is not None:
                        fn()

        def qkv_units(ci):
            return [lambda: qk_part(ci, wq, qt),
                    lambda: qk_part(ci, wk, kt),
                    lambda: v_part(ci)]

        def attn_units(ci):
            return [(lambda hp=hp: attn_part(ci, hp)) for hp in range(HP)]

        def proj_units(ci):
            return [lambda: proj_part(ci, 0), lambda: proj_part(ci, 1)]

        interleave(qkv_units(0))
        if NCH == 1:
            interleave(attn_units(0))
            interleave(proj_units(0))
        else:
            interleave(qkv_units(1))
            for ci in range(NCH - 2):
                streams = [attn_units(ci), qkv_units(ci + 2)]
                if ci >= 1:
                    streams.append(proj_units(ci - 1))
                interleave(*streams)
            interleave(attn_units(NCH - 2),
                       proj_units(NCH - 3) if NCH >= 3 else [])
            interleave(attn_units(NCH - 1),
                       proj_units(NCH - 2) if NCH >= 2 else [])
            ka_ps = mm_ps.tile([128, CH], fp32, tag="mm", name="ka_ps")
            for _ in range(8):
                nc.tensor.matmul(ka_ps[:], warm_w[:],
                                 otn[NCH - 1][:, 0, :], start=True, stop=True)
            interleave(proj_units(NCH - 1))




_CACHE = {}


def _get_nc(S, D, HC):
    key = (S, D, HC)
    if key not in _CACHE:
        nc = bacc.Bacc(None, target_bir_lowering=False)
        build_core(nc, S, D, HC)
        nc.compile()
        _CACHE[key] = nc
    return _CACHE[key]


def make_in_maps(x, rope_cos, rope_sin, W_qkv, W_out, n_cores=8):
    B, S, D = x.shape
    H = 16
    groups = n_cores // B          # head groups per batch
    HC = H // groups               # heads per core
    M = HC * HD
    MT = M // 128
    DT, NCH = D // 128, S // CH
    perm = rope_perm()
    bf16 = ml_dtypes.bfloat16
    cs, sn = rope_tables(np.asarray(rope_cos), np.asarray(rope_sin), S)
    csn = np.stack([cs.reshape(128, NCH * CH), sn.reshape(128, NCH * CH)],
                   axis=1).reshape(128, 2 * NCH * CH)
    in_maps = []
    xtb_cache = {}
    for c in range(n_cores):
        b, g = divmod(c, groups)
        heads = np.arange(g * HC, (g + 1) * HC)
        qcols = np.concatenate([h * HD + perm for h in heads])
        vcols = np.concatenate([2 * D + h * HD + np.arange(HD) for h in heads])
        if b not in xtb_cache:
            xtb_cache[b] = np.ascontiguousarray(
                np.asarray(x[b]).T.reshape(DT, 128, NCH, CH)
                .transpose(2, 1, 0, 3).reshape(NCH, 128, DT * CH)
            ).astype(bf16)

        def wfmt(wcols):
            return np.ascontiguousarray(
                wcols.reshape(DT, 128, M).transpose(1, 0, 2).reshape(128, DT * M)
            ).astype(bf16)

        wo_np = np.ascontiguousarray(
            W_out[g * M:(g + 1) * M, :].reshape(MT, 128, D)
            .transpose(1, 0, 2).reshape(128, MT * D)).astype(bf16)
        in_maps.append({
            "xt": xtb_cache[b],
            "wq": wfmt(W_qkv[:, qcols]),
            "wk": wfmt(W_qkv[:, D + qcols]),
            "wv": wfmt(W_qkv[:, vcols]),
            "wo": wo_np,
            "csn": np.ascontiguousarray(csn).astype(bf16),
        })
    return in_maps


def unshard_out(res, B, S, D, n_cores=8):
    NCH, NT = S // CH, D // 128
    NT2 = NT // 2
    out = np.zeros((B, S, D), np.float32)
    for c in range(n_cores):
        yt = res.results[c]["yt"].astype(np.float32)  # [NCH, 2, 128, NT2*CH]
        ytf = (yt.reshape(NCH, 2, 128, NT2, CH)
               .transpose(1, 3, 2, 0, 4).reshape(D, S))
        out[c // (n_cores // B)] += ytf.T
    return out


def kernel(x, rope_cos, rope_sin, W_qkv, W_out):
    x = np.asarray(x)
    W_qkv = np.asarray(W_qkv)
    W_out = np.asarray(W_out)
    B, S, D = x.shape
    n_cores = 8
    HC = 16 // (n_cores // B)
    in_maps = make_in_maps(x, rope_cos, rope_sin, W_qkv, W_out, n_cores)
    nc = _get_nc(S, D, HC)
    res = run_bass_kernel_spmd(nc, in_maps, list(range(n_cores)))
    return unshard_out(res, B, S, D, n_cores)

